# revision 44
# baseline (speedup 1.0000x reference)
"""DiT block kernel for Trainium2, 8-core data-parallel over batch.

v2 — restructured from the profiled baseline (1311us):
 - HAM-aware emission: phases interleaved so the PE never idles >1us
   (baseline spent 676us at K=4/8 half clock, incl. a 331us cold stretch
   during self-attention).
 - fp8e4m3 DoubleRow matmuls for qkv/proj/cq/ck/cv/cproj (weights scaled
   x64 to dodge subnormals; q/k descale cancels inside qk-rmsnorm, v and
   gate paths descale explicitly). MLP + attention stay bf16 (fp8 there
   breaks the 2e-2 budget; measured on host).
 - Head rmsnorm stats packed into ONE [16, N] psum tile per projection
   (baseline: 32 single-partition ACT chains, ~85us).
 - Softmax denominators via DVE reciprocal; exp reads 2-bank [128,1024]
   PSUM tiles (halves ACT instruction count).
 - RoPE in bf16 with sign-baked sin tables: 4x tensor_copy rotate + 2x
   tensor_tensor muls; gpsimd (Pool) takes the rot*sin multiply.
 - All activations feature-major [D(part), N(free)]; weights host-packed
   to [128, kt, out] (fp8) or W.T (bf16).

Layout/math tricks kept from the baseline: V token-major with a ones
column computing the softmax denominator inside the AV matmul; scores
built transposed; no softmax max-subtraction (|S| <= 8 via Cauchy-Schwarz
after qk-norm); partition reductions/broadcasts via matmuls.
"""
import sys
import numpy as np

sys.path.insert(0, "/opt/trn_rl_repo")

import ml_dtypes

import concourse.bass as bass
import concourse.tile as tile
import concourse.mybir as mybir

P = 128
N = 1024      # tokens
D = 1024      # model dim
H = 16        # heads
HD = 64       # head dim
KV = 128      # context tokens
MH = 4096     # mlp hidden
DT = D // P   # 8 d-tiles
NJ = N // 512 # 2 n-chunks
EPS = 1e-6
WS = 64.0     # fp8 weight scale

F32 = mybir.dt.float32
F32R = mybir.dt.float32r
BF16 = mybir.dt.bfloat16
FP8 = mybir.dt.float8e4
AF = mybir.ActivationFunctionType
ALU = mybir.AluOpType
DR = mybir.MatmulPerfMode.DoubleRow

_CACHE = {}


def split_multi_waits(nc, limit=1):
    """Walrus codegen accepts at most one sync wait per instruction; Tile's
    add_semaphores emits several.  Hoist extras onto same-engine NoOps placed
    immediately before the instruction (per-engine program order preserved)."""
    n_split = 0
    for f in nc.m.functions:
        for bb in f.blocks:
            insns = bb.instructions
            if not any(i.sync_info is not None and len(i.sync_info.on_wait) > limit
                       for i in insns):
                continue
            new = []
            for ins in insns:
                si = ins.sync_info
                if si is not None and len(si.on_wait) > limit:
                    waits = list(si.on_wait)
                    extra, keep = waits[:-limit], waits[-limit:]
                    for w in extra:
                        nop = mybir.InstNoOp(
                            name=nc.get_next_instruction_name(), ins=[], outs=[])
                        nop.engine = ins.engine
                        nop.sync_info = mybir.SyncInfo(on_wait=[w], on_update=[])
                        new.append(nop)
                    ins.sync_info = mybir.SyncInfo(
                        on_wait=keep, on_update=list(si.on_update))
                    n_split += 1
                new.append(ins)
            bb.instructions = new
    return n_split


def build_program(sim_safe=False, reps=1):
    nc = bass.Bass()

    def dram(name, shape, dt, out=False):
        return nc.declare_dram_parameter(name, list(shape), dt, isOutput=out)

    t = dict(
        xT=dram("xT", [D, N], F32),
        colpack=dram("colpack", [P, 81], F32),
        gpack=dram("gpack", [16, 4 * DT * P], BF16),
        opack=dram("opack", [P, DT * 16], BF16),
        ctx8=dram("ctx8", [P, DT, KV], FP8),
        cos2T=dram("cos2T", [P, N], BF16),
        sinS2T=dram("sinS2T", [P, N], BF16),
        adaT=dram("adaT", [D, 7 * D], BF16),
        rowf=dram("rowf", [1, 3 * D], BF16),
        qkvT8=dram("qkvT8", [P, DT, 3 * D], FP8),
        projT8=dram("projT8", [P, DT, D], FP8),
        cqT8=dram("cqT8", [P, DT, D], FP8),
        ckT8=dram("ckT8", [P, DT, D], FP8),
        cvT8=dram("cvT8", [P, DT, D], FP8),
        cprojT8=dram("cprojT8", [P, DT, D], FP8),
        w1T=dram("w1T", [D, MH], BF16),
        w3T=dram("w3T", [D, MH], BF16),
        w2T=dram("w2T", [MH, D], BF16),
        outT=dram("outT", [D, N], F32, out=True),
    )

    with tile.TileContext(nc) as tc:
        if reps > 1:
            with tc.For_i(0, reps):
                _emit(nc, tc, t, sim_safe)
        else:
            _emit(nc, tc, t, sim_safe)

    if not sim_safe:
        split_multi_waits(nc)
    return nc


def _emit(nc, tc, t, sim_safe=False):
    outT = t["outT"]
    dma = nc.sync.dma_start

    def silu(out, in_, tmp_pool, shape):
        if not sim_safe:
            nc.scalar.activation(out, in_, AF.Silu)
        else:
            sg = tmp_pool.tile(shape, F32, tag="sg_tmp", name="sg_tmp")
            nc.scalar.activation(sg, in_, AF.Sigmoid)
            nc.vector.tensor_mul(out, in_, sg)

    from contextlib import ExitStack
    es = ExitStack()
    es2 = ExitStack()   # closed at end of stage 2
    es.enter_context(nc.allow_low_precision(
        reason="bf16/fp8 pipeline by design; validated vs reference"))
    pers = es.enter_context(tc.tile_pool(name="persist", bufs=1))
    vec = es.enter_context(tc.tile_pool(name="vec", bufs=1))

    # small config DMAs first so adaLN/silu_c start before the 4MB x stream
    colpack = vec.tile([P, 81], F32, tag="colpack", name="colpack")
    dma(out=colpack, in_=t["colpack"][:, :])
    cvt = [colpack[:, j:j + 1] for j in range(8)]
    projb_t = [colpack[:, 8 + j:9 + j] for j in range(DT)]
    cprojb_t = [colpack[:, 16 + j:17 + j] for j in range(DT)]
    adab_t = [colpack[:, 24 + j:25 + j] for j in range(56)]
    maskb_t = colpack[:, 80:81]

    rowf = vec.tile([1, 3 * D], BF16, tag="rowf", name="rowf")
    dma(out=rowf, in_=t["rowf"][:, :])
    w_rows = {nm: rowf[:, i * D:(i + 1) * D]
              for i, nm in enumerate(("norm1_w", "normc_w", "norm2_w"))}
    gpack = vec.tile([16, 4 * DT * P], BF16, tag="gpack", name="gpack")
    dma(out=gpack, in_=t["gpack"][:, :])
    opack = vec.tile([P, DT * 16], BF16, tag="opack", name="opack")
    dma(out=opack, in_=t["opack"][:, :])
    # g2[nm][jt]: [16, 128] masked-gamma stationary; contracting against the
    # full [16, *] rstd stats broadcasts gamma[o]*rstd[head(o), :] per jt.
    g2 = {nm: [gpack[:, (i * DT + jt) * P:(i * DT + jt + 1) * P]
               for jt in range(DT)]
          for i, nm in enumerate(("qn", "kn", "cqn", "ckn"))}

    ones_col_bf = vec.tile([P, 1], BF16, tag="ones_col", name="ones_col")
    nc.vector.memset(ones_col_bf, 1.0)
    ones_row = vec.tile([1, HD], BF16, tag="ones_row", name="ones_row")
    nc.vector.memset(ones_row, 1.0)
    ones_row_f = vec.tile([1, HD], F32, tag="ones_row_f", name="ones_row_f")
    nc.vector.memset(ones_row_f, 1.0)
    ones2 = vec.tile([P, 2], BF16, tag="ones2", name="ones2")
    nc.vector.memset(ones2, 0.0)
    nc.vector.memset(ones2[0:HD, 0:1], 1.0)
    nc.vector.memset(ones2[HD:P, 1:2], 1.0)
    eps_col = vec.tile([P, 1], F32, tag="eps_col", name="eps_col")
    nc.vector.memset(eps_col, EPS)

    # ---- residual stream (whole kernel) ----
    xt = []
    for j in range(DT):
        tl = pers.tile([P, N], F32, tag=f"xt{j}", name=f"xt{j}")
        dma(out=tl, in_=t["xT"][j * P:(j + 1) * P, :])
        xt.append(tl)
    hc8 = pers.tile([P, DT, N], FP8, tag="hc8", name="hc8")

    # ---- adaLN: mod = adaT.T @ silu(c) + ada_b, 56 columns [P,1] ----
    mod = [None] * 56
    silu_c = [vec.tile([P, 1], BF16, tag=f"sc{j}", name=f"sc{j}")
              for j in range(DT)]
    for j in range(DT):
        silu(silu_c[j], cvt[j], vec, [P, 1])

    def ada_part(og_range, wbufs=2, psum_pool=None, psum_tag="mps"):
        from contextlib import ExitStack as _ES
        _es = _ES()
        awp = _es.enter_context(tc.tile_pool(name="ada_w", bufs=wbufs))
        aps = (psum_pool if psum_pool is not None else
               _es.enter_context(tc.tile_pool(name="ada_ps", bufs=2, space="PSUM")))
        with _es:
            for og in og_range:
                blks = []
                for kt in range(DT):
                    blk = awp.tile([P, 7 * P], BF16, tag=f"ablk{kt}", name=f"ablk{kt}")
                    dma(out=blk, in_=t["adaT"][kt * P:(kt + 1) * P,
                                               og * 7 * P:(og + 1) * 7 * P])
                    blks.append(blk)
                ps7 = aps.tile([P, 7], F32, tag=psum_tag, name="mps")
                for i in range(7):
                    for kt in range(DT):
                        nc.tensor.matmul(ps7[:, i:i + 1], blks[kt][:, i * P:(i + 1) * P],
                                         silu_c[kt], start=(kt == 0), stop=(kt == DT - 1))
                for i in range(7):
                    ot = og * 7 + i
                    sb = vec.tile([P, 1], F32, tag=f"mod{ot}", name=f"mod{ot}")
                    nc.vector.tensor_add(sb, ps7[:, i:i + 1], adab_t[ot])
                    mod[ot] = sb

    ada_part(range(0, 3))
    sc1p = {"msa": [None] * DT, "mlp": [None] * DT}

    def sc1p_cols(nmq, q):
        for j in range(DT):
            tl = vec.tile([P, 1], F32, tag=f"sc1p_{nmq}{j}", name=f"sc1p_{nmq}{j}")
            nc.vector.tensor_scalar(out=tl, in0=mod[q * 8 + j], scalar1=1.0,
                                    scalar2=None, op0=ALU.add)
            sc1p[nmq][j] = tl

    sc1p_cols("msa", 1)
    pbg, cpbg = [None] * DT, [None] * DT
    g64, cg64 = [None] * DT, [None] * DT

    def late_gate_cols():
        for j in range(DT):
            tl = vec.tile([P, 1], F32, tag=f"pbg{j}", name=f"pbg{j}")
            nc.vector.tensor_mul(tl, projb_t[j], mod[2 * 8 + j])
            pbg[j] = tl
            tl = vec.tile([P, 1], F32, tag=f"g64_{j}", name=f"g64_{j}")
            nc.vector.tensor_scalar(out=tl, in0=mod[2 * 8 + j], scalar1=1.0 / WS,
                                    scalar2=None, op0=ALU.mult)
            g64[j] = tl
            tl = vec.tile([P, 1], F32, tag=f"cpbg{j}", name=f"cpbg{j}")
            nc.vector.tensor_mul(tl, cprojb_t[j], mod[3 * 8 + j])
            cpbg[j] = tl
            tl = vec.tile([P, 1], F32, tag=f"cg64_{j}", name=f"cg64_{j}")
            nc.vector.tensor_scalar(out=tl, in0=mod[3 * 8 + j], scalar1=1.0 / WS,
                                    scalar2=None, op0=ALU.mult)
            cg64[j] = tl
        sc1p_cols("mlp", 5)

    # ---- full-D rmsnorm: dst8 fp8 [P,DT,N] or dst_bf list of bf16 tiles.
    # ps_pool/bc_pool (+tags) let callers share already-open PSUM pools. ----
    def rmsnorm_full(w_name, dst8=None, dst_bf=None, mod_q=None, sh_cols=None,
                     ps_pool=None, ps_tag="ms", bc_pool=None, bc_tag="bc"):
        from contextlib import ExitStack as _ES
        _es = _ES()
        tmp = _es.enter_context(tc.tile_pool(name="rn_tmp", bufs=2))
        rps = (ps_pool if ps_pool is not None else
               _es.enter_context(tc.tile_pool(name="rn_ps", bufs=1, space="PSUM")))
        bps = (bc_pool if bc_pool is not None else
               _es.enter_context(tc.tile_pool(name="rn_bc", bufs=2, space="PSUM")))
        with _es:
            w_row = w_rows[w_name]
            ms = rps.tile([1, N], F32, tag=ps_tag, name="ms")
            for nj in range(NJ):
                nsl = slice(nj * 512, (nj + 1) * 512)
                for j in range(DT):
                    sq = tmp.tile([P, 512], BF16, tag="rnsq", name="rnsq")
                    nc.scalar.activation(sq, xt[j][:, nsl], AF.Square)
                    nc.tensor.matmul(ms[:, nsl], ones_col_bf, sq,
                                     start=(j == 0), stop=(j == DT - 1))
                lnv = tmp.tile([1, 512], F32, tag="rnlnv", name="rnlnv")
                nc.scalar.activation(lnv, ms[:, nsl], AF.Ln, bias=eps_col[0:1, :],
                                     scale=1.0 / D)
                rstd = tmp.tile([1, 512], BF16, tag="rnrstd", name="rnrstd")
                nc.scalar.activation(rstd, lnv, AF.Exp, scale=-0.5)
                for j in range(DT):
                    bc = bps.tile([P, 512], F32, tag=bc_tag, name="rnbc")
                    nc.tensor.matmul(bc, w_row[:, j * P:(j + 1) * P],
                                     rstd, start=True, stop=True)
                    if mod_q is None:
                        nc.vector.tensor_mul(dst8[:, j, nsl], xt[j][:, nsl], bc)
                    else:
                        xn = tmp.tile([P, 512], BF16, tag="rnxn", name="rnxn")
                        nc.vector.tensor_mul(xn, xt[j][:, nsl], bc)
                        out_ap = (dst8[:, j, nsl] if dst8 is not None
                                  else dst_bf[j][:, nsl])
                        nc.gpsimd.tensor_scalar(out=out_ap, in0=xn,
                                                scalar1=sc1p[mod_q][j],
                                                scalar2=sh_cols[j],
                                                op0=ALU.mult, op1=ALU.add)

    # ---- fp8 DR matmul helper: out_ps[:, csl] += wsec[:,2t:2t+2, osl] . src8 ----
    def dr_mms(ps, wsec_ap_fn, src8, nsl):
        for tt in range(DT // 2):
            nc.tensor.matmul(ps, wsec_ap_fn(tt),
                             src8[:, 2 * tt:2 * tt + 2, nsl],
                             start=(tt == 0), stop=(tt == DT // 2 - 1),
                             perf_mode=DR)

    # ---- packed head-norm stats: hs [16, W] psum -> rstd16 [16, W] bf16 ----
    def head_stats(hs, rstd16, W):
        lnv = vec.tile([16, N], F32, tag="hslnv", name="hslnv")
        nc.scalar.activation(lnv[:, 0:W], hs[:, 0:W], AF.Ln,
                             bias=eps_col[0:16, :], scale=1.0 / HD)
        nc.scalar.activation(rstd16[:, 0:W], lnv[:, 0:W], AF.Exp, scale=-0.5)

    # =========== stage 1: self-attention superblock ===========
    wc = es2.enter_context(tc.tile_pool(name="wc", bufs=1))
    with tc.tile_pool(name="s1", bufs=1) as s1:
        cos2 = s1.tile([P, N], BF16, tag="cos2", name="cos2")
        sinS = s1.tile([P, N], BF16, tag="sinS", name="sinS")
        dma(out=cos2, in_=t["cos2T"][:, :])
        dma(out=sinS, in_=t["sinS2T"][:, :])
        vstore = [s1.tile([P, H, HD + 1], BF16, tag=f"v{j}", name=f"v{j}")
                  for j in range(DT)]
        # qsb/ksb hold the raw projections, then are overwritten in place by
        # the roped+normalized q-hat/k-hat (WAR deps only).
        qsb = [s1.tile([P, N], BF16, tag=f"qsb{j}", name=f"qsb{j}")
               for j in range(DT)]
        ksb = [s1.tile([P, N], BF16, tag=f"ksb{j}", name=f"ksb{j}")
               for j in range(DT)]
        qhat, khat = qsb, ksb
        rstdq = s1.tile([16, N], BF16, tag="rstdq", name="rstdq")
        rstdk = s1.tile([16, N], BF16, tag="rstdk", name="rstdk")

        with tc.tile_pool(name="h8p", bufs=1) as h8p, \
             tc.tile_pool(name="wqkv", bufs=2) as wq, \
             tc.tile_pool(name="qk_mm", bufs=2, space="PSUM") as mmp, \
             tc.tile_pool(name="qk_hs", bufs=1, space="PSUM") as hsp, \
             tc.tile_pool(name="qk_vps", bufs=2, space="PSUM") as vpsp, \
             tc.tile_pool(name="qk_tmp", bufs=3) as tmp:

            def load_sec(sec):
                tl = wq.tile([P, DT, 1024], FP8, tag="qkvsec", name="qkvsec")
                dma(out=tl, in_=t["qkvT8"][:, :, sec * 1024:(sec + 1) * 1024])
                return tl

            h8 = h8p.tile([P, DT, N], FP8, tag="h8", name="h8")
            rmsnorm_full("norm1_w", dst8=h8, mod_q="msa", sh_cols=mod[0:8],
                         ps_pool=mmp, ps_tag="qkmm", bc_pool=vpsp, bc_tag="vmm")

            qsec = load_sec(0)
            ksec = load_sec(1)

            # ---- Q/K projections + packed stats ----
            def qk_proj(sec_tile, jt, sb_list, hs_ps):
                ps = mmp.tile([P, N], F32, tag="qkmm", name="qkmm")
                for c in range(2):
                    dr_mms(ps[:, c * 512:(c + 1) * 512],
                           lambda tt: sec_tile[:, 2 * tt:2 * tt + 2,
                                               jt * P:(jt + 1) * P],
                           h8, slice(c * 512, (c + 1) * 512))
                nc.scalar.copy(sb_list[jt], ps)
                sq = tmp.tile([P, N], BF16, tag="qksq", name="qksq")
                nc.scalar.activation(sq, ps, AF.Square)
                for c in range(2):
                    nc.tensor.matmul(hs_ps[:, c * 512:(c + 1) * 512],
                                     opack[:, jt * 16:(jt + 1) * 16],
                                     sq[:, c * 512:(c + 1) * 512],
                                     start=(jt == 0), stop=(jt == DT - 1))

            hsq = hsp.tile([16, N], F32, tag="hsq", name="hsq")
            for jt in range(DT):
                qk_proj(qsec, jt, qsb, hsq)
            head_stats(hsq, rstdq, N)
            vsec = load_sec(2)
            # K reuses the hsq bank; its writes wait only on the one ACT read
            # of the Q stats (WAR handled by Tile's bank tracker).
            for jt in range(DT):
                qk_proj(ksec, jt, ksb, hsq)
            head_stats(hsq, rstdk, N)

            # ---- V build (token-major, ones col interleaved); PE work here
            # covers the rope/finish DVE phase that follows ----
            for mt in range(DT):
                nc.vector.memset(vstore[mt][:, :, HD:HD + 1], 1.0)
                for vj in range(2):
                    ps = vpsp.tile([P, 512], F32, tag="vmm", name="vmm")
                    dr_mms(ps, lambda tt: h8[:, 2 * tt:2 * tt + 2,
                                             mt * P:(mt + 1) * P],
                           vsec, slice(vj * 512, (vj + 1) * 512))
                    nc.vector.tensor_scalar(
                        out=vstore[mt][:, vj * 8:(vj + 1) * 8, 0:HD],
                        in0=ps,
                        scalar1=1.0 / WS, scalar2=None, op0=ALU.mult)


        # ---- self-attention; finishes interleaved with heads ----
        o8 = s1.tile([P, DT, N], FP8, tag="o8", name="o8")
        wproj = s1.tile([P, DT, D], FP8, tag="wproj", name="wproj")
        dma(out=wproj, in_=t["projT8"][:, :, :])
        # prefetch stage-2 weights during the ACT-bound attention window
        wcq = wc.tile([P, DT, D], FP8, tag="wcq", name="wcq")
        wck = wc.tile([P, DT, D], FP8, tag="wck", name="wck")
        wcv = wc.tile([P, DT, D], FP8, tag="wcv", name="wcv")
        wcproj = wc.tile([P, DT, D], FP8, tag="wcproj", name="wcproj")
        dma(out=wcq, in_=t["cqT8"][:, :, :])
        dma(out=wck, in_=t["ckT8"][:, :, :])
        dma(out=wcv, in_=t["cvT8"][:, :, :])
        dma(out=wcproj, in_=t["cprojT8"][:, :, :])

        with tc.tile_pool(name="at_s", bufs=2, space="PSUM") as sps, \
             tc.tile_pool(name="at_o", bufs=2, space="PSUM") as ops, \
             tc.tile_pool(name="at_f", bufs=2, space="PSUM") as fbp, \
             tc.tile_pool(name="at_e", bufs=2) as ep, \
             tc.tile_pool(name="at_r", bufs=2) as rp:

            def qk_finish(jt, sb_list, rstd16, grow, rope):
                bq_sb = rp.tile([P, N], BF16, tag="bqsb", name="bqsb")
                for c in range(2):
                    bq = fbp.tile([P, 512], F32, tag="fbq", name="fbq")
                    nc.tensor.matmul(bq, g2[grow][jt],
                                     rstd16[:, c * 512:(c + 1) * 512],
                                     start=True, stop=True)
                    nc.scalar.copy(bq_sb[:, c * 512:(c + 1) * 512], bq)
                src = sb_list[jt]
                if rope:
                    rot = rp.tile([P, N], BF16, tag="rtmp", name="rot")
                    for half in range(2):
                        b = half * HD
                        nc.vector.tensor_copy(rot[b:b + 32, :], src[b + 32:b + 64, :])
                        nc.vector.tensor_copy(rot[b + 32:b + 64, :], src[b:b + 32, :])
                    rs = rp.tile([P, N], BF16, tag="rtmp", name="rs")
                    nc.vector.tensor_mul(rs, rot, sinS)
                    m1 = rp.tile([P, N], BF16, tag="rtmp", name="m1")
                    nc.vector.tensor_mul(m1, src, cos2)
                    nc.vector.tensor_add(m1, m1, rs)
                    nc.vector.tensor_mul(src, m1, bq_sb)
                else:
                    nc.vector.tensor_mul(src, src, bq_sb)

            def attn_head(hh, nj, qsrc, ksrc, vtiles, n_mt, dst8, mask_col):
                jt, half = hh // 2, hh % 2
                hsl = slice(half * HD, (half + 1) * HD)
                nsl = slice(nj * 512, (nj + 1) * 512)
                o_ps = ops.tile([HD + 1, 512], F32, tag="ops", name="ops")
                for pr in range(n_mt // 2):
                    # two key-blocks share a 2-bank psum tile -> one wide exp
                    s_ps = sps.tile([P, 1024], F32, tag="sps", name="sps")
                    for w in range(2):
                        mt = 2 * pr + w
                        nc.tensor.matmul(s_ps[:, w * 512:(w + 1) * 512],
                                         ksrc[jt][hsl, mt * P:(mt + 1) * P],
                                         qsrc[jt][hsl, nsl],
                                         start=True, stop=True)
                    e_bf = ep.tile([P, 1024], BF16, tag="ebf", name="ebf")
                    nc.scalar.activation(e_bf, s_ps, AF.Exp)
                    for w in range(2):
                        mt = 2 * pr + w
                        nc.tensor.matmul(o_ps, vtiles[mt][:, hh, :],
                                         e_bf[:, w * 512:(w + 1) * 512],
                                         start=(mt == 0), stop=(mt == n_mt - 1))
                # slow DVE iterative reciprocal, but off the PE path: 4 o_ps
                # bufs let later heads' matmuls run during this tail
                r_b = rp.tile([1, 512], BF16, tag="rb", name="rb")
                nc.vector.reciprocal(r_b, o_ps[HD:HD + 1, :])
                br = fbp.tile([HD, 512], F32, tag="fbq", name="br")
                nc.tensor.matmul(br, ones_row, r_b, start=True, stop=True)
                br_sb = rp.tile([HD, 512], BF16, tag="brsb", name="brsb")
                nc.scalar.copy(br_sb, br)
                nc.vector.tensor_mul(dst8[hsl, jt, nsl], o_ps[0:HD, :], br_sb)

            for jt in range(DT):
                if jt == 1:
                    # adaLN og3-7: PE/DMA work filling the ACT-bound window
                    ada_part(range(3, 8), wbufs=1, psum_pool=fbp,
                             psum_tag="fbq")
                    late_gate_cols()
                qk_finish(jt, qsb, rstdq, "qn", True)
                qk_finish(jt, ksb, rstdk, "kn", True)
                for half in range(2):
                    for nj in range(NJ):
                        attn_head(2 * jt + half, nj, qhat, khat, vstore, DT,
                                  o8, None)

        # ---- proj + gated residual + rmsnorm_c, chunk-interleaved ----
        with tc.tile_pool(name="pr_x", bufs=2, space="PSUM") as xps, \
             tc.tile_pool(name="pr_m", bufs=1, space="PSUM") as msp, \
             tc.tile_pool(name="pr_t", bufs=3) as rp:
            for nj in range(NJ):
                nsl = slice(nj * 512, (nj + 1) * 512)
                for ot in range(DT):
                    ps = xps.tile([P, 512], F32, tag="prj", name="prj")
                    dr_mms(ps, lambda tt: wproj[:, 2 * tt:2 * tt + 2,
                                                ot * P:(ot + 1) * P],
                           o8, nsl)
                    tsb = rp.tile([P, 512], F32, tag="tsb", name="tsb")
                    nc.scalar.activation(tsb, ps, AF.Identity,
                                         bias=pbg[ot], scale=g64[ot])
                    nc.vector.tensor_add(xt[ot][:, nsl], xt[ot][:, nsl], tsb)
                # rmsnorm_c for this chunk -> hc8 (DVE squares; ACT stats)
                ms = msp.tile([1, 512], F32, tag="msc", name="msc")
                for j in range(DT):
                    sq = rp.tile([P, 512], BF16, tag="rnsqc", name="rnsqc")
                    nc.vector.tensor_mul(sq, xt[j][:, nsl], xt[j][:, nsl])
                    nc.tensor.matmul(ms, ones_col_bf, sq,
                                     start=(j == 0), stop=(j == DT - 1))
                lnv = rp.tile([1, 512], F32, tag="rnlnvc", name="rnlnvc")
                nc.scalar.activation(lnv, ms, AF.Ln, bias=eps_col[0:1, :],
                                     scale=1.0 / D)
                rstd = rp.tile([1, 512], BF16, tag="rnrstdc", name="rnrstdc")
                nc.scalar.activation(rstd, lnv, AF.Exp, scale=-0.5)
                for j in range(DT):
                    bc = xps.tile([P, 512], F32, tag="prj", name="bcc")
                    nc.tensor.matmul(bc, w_rows["normc_w"][:, j * P:(j + 1) * P],
                                     rstd, start=True, stop=True)
                    nc.vector.tensor_mul(hc8[:, j, nsl], xt[j][:, nsl], bc)

    # =========== stage 2: cross-attention ===========
    with tc.tile_pool(name="s4", bufs=1) as s4:
        ctx8 = s4.tile([P, DT, KV], FP8, tag="ctx8", name="ctx8")
        dma(out=ctx8, in_=t["ctx8"][:, :, :])
        qc_hat = [s4.tile([P, N], BF16, tag=f"qch{j}", name=f"qch{j}")
                  for j in range(DT)]
        kc_hat = [s4.tile([P, KV], BF16, tag=f"kch{j}", name=f"kch{j}")
                  for j in range(DT)]
        vc_store = s4.tile([KV, H, HD + 1], BF16, tag="vc", name="vc")
        rstdcq = s4.tile([16, N], BF16, tag="rstdcq", name="rstdcq")
        rstdck = s4.tile([16, KV], BF16, tag="rstdck", name="rstdck")

        with tc.tile_pool(name="cq_mm", bufs=2, space="PSUM") as mmp, \
             tc.tile_pool(name="cq_hs", bufs=1, space="PSUM") as hsp, \
             tc.tile_pool(name="cq_bk", bufs=2, space="PSUM") as bkp, \
             tc.tile_pool(name="cq_tmp", bufs=3) as tmp:
            # K_c: per jt [P, KV]; raw k evacuated to SBUF bf16 (kc_hat, then
            # normalized in place)
            hsk = hsp.tile([16, KV], F32, tag="hs", name="hsk")
            for jt in range(DT):
                ps = mmp.tile([P, KV], F32, tag="cmm", name="ckmm")
                dr_mms(ps,
                       lambda tt: wck[:, 2 * tt:2 * tt + 2, jt * P:(jt + 1) * P],
                       ctx8, slice(0, KV))
                nc.scalar.copy(kc_hat[jt], ps)
                sq = tmp.tile([P, KV], BF16, tag="csq", name="csq")
                nc.scalar.activation(sq, ps, AF.Square)
                nc.tensor.matmul(hsk, opack[:, jt * 16:(jt + 1) * 16], sq,
                                 start=(jt == 0), stop=(jt == DT - 1))
            head_stats(hsk, rstdck, KV)
            for jt in range(DT):
                bk = bkp.tile([P, KV], F32, tag="bk", name="bk")
                nc.tensor.matmul(bk, g2["ckn"][jt], rstdck,
                                 start=True, stop=True)
                bk_sb = tmp.tile([P, KV], BF16, tag="bksb", name="bksb")
                nc.scalar.copy(bk_sb, bk)
                nc.vector.tensor_mul(kc_hat[jt], kc_hat[jt], bk_sb)
            # V_c (token-major)
            nc.vector.memset(vc_store[:, :, HD:HD + 1], 1.0)
            for vj in range(2):
                ps = mmp.tile([P, N], F32, tag="cmm", name="cmm")
                for tt in range(DT // 2):
                    nc.tensor.matmul(ps[:, vj * 512:(vj + 1) * 512],
                                     ctx8[:, 2 * tt:2 * tt + 2, :],
                                     wcv[:, 2 * tt:2 * tt + 2,
                                         vj * 512:(vj + 1) * 512],
                                     start=(tt == 0), stop=(tt == DT // 2 - 1),
                                     perf_mode=DR)
                nc.vector.tensor_scalar(
                    out=vc_store[:, vj * 8:(vj + 1) * 8, 0:HD],
                    in0=ps[:, vj * 512:(vj + 1) * 512],
                    scalar1=1.0 / WS, scalar2=None, op0=ALU.mult)
            # Q_c: like q but no rope; raw q evacuated into qc_hat (bf16),
            # then normalized in place
            hsq2 = hsp.tile([16, N], F32, tag="hs", name="hsq2")
            for jt in range(DT):
                ps = mmp.tile([P, N], F32, tag="cmm", name="cqmm")
                for c in range(2):
                    dr_mms(ps[:, c * 512:(c + 1) * 512],
                           lambda tt: wcq[:, 2 * tt:2 * tt + 2,
                                          jt * P:(jt + 1) * P],
                           hc8, slice(c * 512, (c + 1) * 512))
                nc.scalar.copy(qc_hat[jt], ps)
                sq = tmp.tile([P, N], BF16, tag="cqsq", name="cqsq")
                nc.scalar.activation(sq, ps, AF.Square)
                for c in range(2):
                    nc.tensor.matmul(hsq2[:, c * 512:(c + 1) * 512],
                                     opack[:, jt * 16:(jt + 1) * 16],
                                     sq[:, c * 512:(c + 1) * 512],
                                     start=(jt == 0), stop=(jt == DT - 1))
            head_stats(hsq2, rstdcq, N)
            for jt in range(DT):
                bq = mmp.tile([P, N], F32, tag="cmm", name="cqmm")
                for c in range(2):
                    nc.tensor.matmul(bq[:, c * 512:(c + 1) * 512], g2["cqn"][jt],
                                     rstdcq[:, c * 512:(c + 1) * 512],
                                     start=True, stop=True)
                bq_sb = tmp.tile([P, N], BF16, tag="bqsb2", name="bqsb2")
                nc.scalar.copy(bq_sb, bq)
                nc.vector.tensor_mul(qc_hat[jt], qc_hat[jt], bq_sb)

        with tc.tile_pool(name="at2_s", bufs=2, space="PSUM") as sps, \
             tc.tile_pool(name="at2_o", bufs=2, space="PSUM") as ops, \
             tc.tile_pool(name="at2_b", bufs=1, space="PSUM") as bps, \
             tc.tile_pool(name="at2_x", bufs=1, space="PSUM") as xps, \
             tc.tile_pool(name="at2_e", bufs=4) as ep, \
             tc.tile_pool(name="at2_r", bufs=3) as rp:
            o8c = s4.tile([P, DT, N], FP8, tag="o8c", name="o8c")

            for nj in range(NJ):
                for hh in range(H):
                    jt, half = hh // 2, hh % 2
                    hsl = slice(half * HD, (half + 1) * HD)
                    nsl = slice(nj * 512, (nj + 1) * 512)
                    o_ps = ops.tile([HD + 1, 512], F32, tag="ops2", name="ops2")
                    s_ps = sps.tile([P, 512], F32, tag="sps2", name="sps2")
                    nc.tensor.matmul(s_ps, kc_hat[jt][hsl, :],
                                     qc_hat[jt][hsl, nsl], start=True, stop=True)
                    e_bf = ep.tile([P, 512], BF16, tag="ebf2", name="ebf2")
                    nc.scalar.activation(e_bf, s_ps, AF.Exp, bias=maskb_t)
                    nc.tensor.matmul(o_ps, vc_store[:, hh, :],
                                     e_bf, start=True, stop=True)
                    r_f = rp.tile([1, 512], F32, tag="rf2", name="rf2")
                    nc.scalar.activation(r_f, o_ps[HD:HD + 1, :], AF.Ln)
                    r_b = rp.tile([1, 512], BF16, tag="rb2", name="rb2")
                    nc.scalar.activation(r_b, r_f, AF.Exp, scale=-1.0)
                    br = bps.tile([HD, 512], F32, tag="br2", name="br2")
                    nc.tensor.matmul(br, ones_row, r_b, start=True, stop=True)
                    br_sb = rp.tile([HD, 512], BF16, tag="brsb2", name="brsb2")
                    nc.scalar.copy(br_sb, br)
                    nc.vector.tensor_mul(o8c[hsl, jt, nsl], o_ps[0:HD, :], br_sb)
                # cproj for this chunk
                nsl = slice(nj * 512, (nj + 1) * 512)
                for ot in range(DT):
                    ps = xps.tile([P, 512], F32, tag="cprj", name="cprj")
                    dr_mms(ps, lambda tt: wcproj[:, 2 * tt:2 * tt + 2,
                                                 ot * P:(ot + 1) * P],
                           o8c, nsl)
                    tsb = rp.tile([P, 512], F32, tag="tsb2", name="tsb2")
                    nc.scalar.activation(tsb, ps, AF.Identity,
                                         bias=cpbg[ot], scale=cg64[ot])
                    nc.vector.tensor_add(xt[ot][:, nsl], xt[ot][:, nsl], tsb)

    es2.close()

    # =========== stage 3: SwiGLU MLP ===========
    with tc.tile_pool(name="gpool", bufs=1) as gp:
        g_tiles = [gp.tile([P, N], BF16, tag=f"g{mt}", name=f"g{mt}")
                   for mt in range(MH // P)]
        with tc.tile_pool(name="hbp", bufs=1) as hbp, \
             tc.tile_pool(name="wmlp", bufs=2) as wp, \
             tc.tile_pool(name="ml_t", bufs=3) as tp, \
             tc.tile_pool(name="ml_ps", bufs=2, space="PSUM") as mps:
            h_bf = [hbp.tile([P, N], BF16, tag=f"hb{j}", name=f"hb{j}")
                    for j in range(DT)]
            rmsnorm_full("norm2_w", dst_bf=h_bf, mod_q="mlp",
                         sh_cols=mod[4 * 8:5 * 8])
            for c in range(MH // 512):
                w1c, w3c = [], []
                for kt in range(DT):
                    tl = wp.tile([P, 512], BF16, tag=f"w1_{kt}", name=f"w1_{kt}")
                    dma(out=tl, in_=t["w1T"][kt * P:(kt + 1) * P, c * 512:(c + 1) * 512])
                    w1c.append(tl)
                    tl = wp.tile([P, 512], BF16, tag=f"w3_{kt}", name=f"w3_{kt}")
                    dma(out=tl, in_=t["w3T"][kt * P:(kt + 1) * P, c * 512:(c + 1) * 512])
                    w3c.append(tl)
                for i in range(4):
                    mt = c * 4 + i
                    isl = slice(i * P, (i + 1) * P)
                    for nj in range(NJ):
                        nsl = slice(nj * 512, (nj + 1) * 512)
                        ups = mps.tile([P, 512], F32, tag="ups", name="ups")
                        for kt in range(DT):
                            nc.tensor.matmul(ups, w1c[kt][:, isl], h_bf[kt][:, nsl],
                                             start=(kt == 0), stop=(kt == DT - 1))
                        su = tp.tile([P, 512], BF16, tag="su", name="su")
                        silu(su, ups, tp, [P, 512])
                        tps = mps.tile([P, 512], F32, tag="tps", name="tps")
                        for kt in range(DT):
                            nc.tensor.matmul(tps, w3c[kt][:, isl], h_bf[kt][:, nsl],
                                             start=(kt == 0), stop=(kt == DT - 1))
                        nc.vector.tensor_mul(g_tiles[mt][:, nsl], su, tps)

        with tc.tile_pool(name="w2p", bufs=1) as w2p, \
             tc.tile_pool(name="fin_t", bufs=3) as tp, \
             tc.tile_pool(name="fin_ps", bufs=2, space="PSUM") as fps:
            w2_sb = []
            for mt in range(MH // P):
                tl = w2p.tile([P, D], BF16, tag=f"w2_{mt}", name=f"w2_{mt}")
                dma(out=tl, in_=t["w2T"][mt * P:(mt + 1) * P, :])
                w2_sb.append(tl)
            for ot in range(DT):
                for nj in range(NJ):
                    nsl = slice(nj * 512, (nj + 1) * 512)
                    ps = fps.tile([P, 512], F32, tag="yps", name="yps")
                    for mt in range(MH // P):
                        nc.tensor.matmul(ps, w2_sb[mt][:, ot * P:(ot + 1) * P],
                                         g_tiles[mt][:, nsl],
                                         start=(mt == 0), stop=(mt == MH // P - 1))
                    tsb = tp.tile([P, 512], F32, tag="t", name="t")
                    nc.scalar.activation(tsb, ps, AF.Identity, bias=0.0,
                                         scale=mod[6 * 8 + ot])
                    osb = tp.tile([P, 512], F32, tag="os", name="os")
                    nc.vector.tensor_add(osb, xt[ot][:, nsl], tsb)
                    dma(out=outT[ot * P:(ot + 1) * P, nsl], in_=osb)

    es.close()


# ===================== host side =====================

def _prep_inputs(inputs):
    bf = ml_dtypes.bfloat16
    f8 = ml_dtypes.float8_e4m3
    f32 = np.float32

    def wT(a):
        return np.ascontiguousarray(np.asarray(a, f32).T).astype(bf)

    def w8(a):
        # W.T scaled x WS, fp8, repacked [in,out] -> [128, kt, out]
        wt = np.asarray(a, f32).T * WS
        kt = wt.shape[0] // P
        return np.ascontiguousarray(
            wt.reshape(kt, P, -1).transpose(1, 0, 2)).astype(f8)

    x = np.asarray(inputs["x"], f32)
    c = np.asarray(inputs["c"], f32)
    context = np.asarray(inputs["context"], f32)
    mask = np.asarray(inputs["context_mask"]).astype(bool)
    cos = np.asarray(inputs["rope_cos"], f32)   # [N, HD]
    sin = np.asarray(inputs["rope_sin"], f32)

    scale = HD ** -0.5
    rowf = np.concatenate([np.asarray(inputs[nm], f32).reshape(-1)
                           for nm in ("norm1_w", "normc_w", "norm2_w")]
                          ).reshape(1, -1).astype(bf)

    # gpack [16, 4*8*128]: per (proj, jt) a [16, 128] masked-gamma block.
    # Row r nonzero only where r == 2*jt + (o >= 64); value gamma[o % 64].
    gpack = np.zeros((16, 4 * DT * P), f32)
    for i, (nm, s) in enumerate((("qn_w", scale), ("kn_w", 1.0),
                                 ("cqn_w", scale), ("ckn_w", 1.0))):
        g = np.asarray(inputs[nm], f32) * s  # [HD]
        for jt in range(DT):
            base = (i * DT + jt) * P
            gpack[2 * jt, base:base + HD] = g
            gpack[2 * jt + 1, base + HD:base + P] = g
    gpack = gpack.astype(bf)

    # opack [128, jt*16+r]: 1 iff r == 2*jt + (p >= 64); head-sum masks
    opack = np.zeros((P, DT * 16), f32)
    for jt in range(DT):
        opack[0:HD, jt * 16 + 2 * jt] = 1.0
        opack[HD:P, jt * 16 + 2 * jt + 1] = 1.0
    opack = opack.astype(bf)

    cos2 = np.concatenate([cos.T, cos.T], axis=0)          # [128, N]
    sinS = np.concatenate([sin.T, sin.T], axis=0).copy()   # [128, N]
    for b in range(2):
        sinS[b * HD:b * HD + 32, :] *= -1.0

    shared = {
        "cos2T": np.ascontiguousarray(cos2).astype(bf),
        "sinS2T": np.ascontiguousarray(sinS).astype(bf),
        "adaT": wT(inputs["ada_w"]),
        "rowf": rowf,
        "gpack": np.ascontiguousarray(gpack),
        "opack": np.ascontiguousarray(opack),
        "qkvT8": w8(inputs["qkv_w"]),
        "projT8": w8(inputs["proj_w"]),
        "cqT8": w8(inputs["cq_w"]),
        "ckT8": w8(inputs["ck_w"]),
        "cvT8": w8(inputs["cv_w"]),
        "cprojT8": w8(inputs["cproj_w"]),
        "w1T": wT(inputs["w1"]),
        "w3T": wT(inputs["w3"]),
        "w2T": wT(inputs["w2"]),
    }
    proj_b = np.asarray(inputs["proj_b"], f32).reshape(8, P).T       # [P, 8]
    cproj_b = np.asarray(inputs["cproj_b"], f32).reshape(8, P).T
    ada_b = np.asarray(inputs["ada_b"], f32).reshape(56, P).T        # [P, 56]

    in_maps = []
    for b in range(8):
        m = mask[b].copy()
        if not m.any():
            m[0] = True
        maskb = np.where(m, 0.0, -60.0).astype(f32).reshape(-1, 1)
        colpack = np.concatenate([
            c[b].reshape(8, P).T, proj_b, cproj_b, ada_b, maskb], axis=1)
        ctx8 = np.ascontiguousarray(
            context[b].T.reshape(DT, P, KV).transpose(1, 0, 2)).astype(f8)
        per = {
            "xT": np.ascontiguousarray(x[b].T),
            "colpack": np.ascontiguousarray(colpack.astype(f32)),
            "ctx8": ctx8,
        }
        per.update(shared)
        in_maps.append(per)
    return in_maps


def get_program():
    if "nc" not in _CACHE:
        _CACHE["nc"] = build_program()
    return _CACHE["nc"]


def kernel(**inputs):
    from concourse.bass_utils import run_bass_kernel_spmd
    nc = get_program()
    in_maps = _prep_inputs(inputs)
    res = run_bass_kernel_spmd(nc, in_maps, list(range(8)), trace=False)
    out = np.empty((8, N, D), np.float32)
    for b in range(8):
        out[b] = res.results[b]["outT"].T
    return out
